# revision 18
# baseline (speedup 1.0000x reference)
"""Trainium2 Bass kernel for nn_HaarBlock (binary CNN block).

Reference computation (per image):
    x  = sign1(x - mean(x_full))                                  # global mean!
    y1 = sign1(depthwise_conv7x7(x, sign1(w1)) + round(b1))       # pad 3, same
    y2 = sign1(conv1x1(y1, sign1(w2)) + round(b2))                # pad 3 (!) -> 390x390
    y3 = sign1(conv1x1(y2, sign1(w3)) + round(b3))                # stride 3, pad 1 -> 131x131

Key structural facts exploited here (sign1(v) = +1 if v >= 0 else -1):
  * conv3 (1x1, stride 3, pad 1) samples y2 only at rows/cols 3q-1.  Valid
    y1 data reaches those samples only for q in [2,129] -> a 128x128 interior
    grid; y1 is needed only at rows/cols {2,5,...,383} (128 strided positions).
  * Output border ring is constant: row/col 0 = sign1(rb3) (conv3 zero pad),
    ring q in {1,130} = sign1(W3b @ sign1(rb2) + rb3) (conv2 zero pad).
  * All weights are +-1, so the depthwise conv decomposes into matmuls with
    shift matrices whose diagonal values are the tap signs.  Per (channel,
    dx, row-phase) the 2-3 taps collapse into one of only 12 distinct
    sign-combo shift matrices (host-built, shared across all channels).

Data-parallel over batch: 8 images -> 8 NeuronCores, no cross-core comms
(the global mean is a host-side scalar reduction).
"""

import numpy as np

C_IN, C_OUT, K = 14, 128, 7
H = W = 384
NCORES = 8
QO = 131          # output spatial size
NG = 128          # computed interior grid (qi, qj in [2,129] of the 131 grid)
PIX = NG * NG     # 16384
BLK = 512         # pixels per conv2/conv3 block (= 4 output rows)
NBLK = PIX // BLK

# dy -> (phase phi of input row 3*qi+2+dy, shift j so that row index m = qi+j)
#   row = 3*qi+2+dy = 3*(qi+j) + phi
_DY_MAP = {-3: (2, -1), -2: (0, 0), -1: (1, 0), 0: (2, 0),
           1: (0, 1), 2: (1, 1), 3: (2, 1)}
# slot -> (phase, [dy list]) : slot 0/1 are 2-tap (shifts 0,+1), slot 2 is
# 3-tap (shifts -1,0,+1)
_SLOTS = [(0, [-2, 1]), (1, [-1, 2]), (2, [-3, 0, 3])]
# dx -> valid qj range [qj0, qj1) such that tap col 3*qj+2+dx is in [0, 384)
_DX_QJ = {-3: (1, 128), -2: (0, 128), -1: (0, 128), 0: (0, 128),
          1: (0, 127), 2: (0, 127), 3: (0, 127)}
_DX_ORDER = [-2, -1, 0, -3, 1, 2, 3]   # first one must span the full qj range


def _sign1(a):
    return np.where(a >= 0.0, 1.0, -1.0).astype(np.float32)


def _build_combos():
    """12 stationary matrices [m, qi] in bf16:
    idx 0..3:  sa*[m==qi] + sb*[m==qi+1]          (2-tap, bits: sa,sb)
    idx 4..11: sa*[m==qi-1] + sb*[m==qi] + sc*[m==qi+1]  (3-tap, bits: sa,sb,sc)
    """
    import ml_dtypes
    eye = np.eye(NG, dtype=np.float32)                      # [m, qi] m==qi
    up = np.zeros((NG, NG), dtype=np.float32)               # m == qi+1
    up[1:, :-1] = np.eye(NG - 1, dtype=np.float32)
    dn = np.zeros((NG, NG), dtype=np.float32)               # m == qi-1
    dn[:-1, 1:] = np.eye(NG - 1, dtype=np.float32)
    out = np.zeros((12, NG, NG), dtype=np.float32)
    for b in range(4):
        sa = 1.0 if (b >> 1) & 1 else -1.0
        sb = 1.0 if b & 1 else -1.0
        out[b] = sa * eye + sb * up
    for b in range(8):
        sa = 1.0 if (b >> 2) & 1 else -1.0
        sb = 1.0 if (b >> 1) & 1 else -1.0
        sc = 1.0 if b & 1 else -1.0
        out[4 + b] = sa * dn + sb * eye + sc * up
    return out.astype(ml_dtypes.bfloat16)


def _combo_idx(s, c, slot, dx):
    """Combo tile index for channel c, slot, dx.  s = sign1(w1)[c, dy+3, dx+3]."""
    _, dys = _SLOTS[slot]
    bits = [s[c, dy + 3, dx + 3] > 0 for dy in dys]
    if slot < 2:
        return (bits[0] << 1) | bits[1]
    return 4 + ((bits[0] << 2) | (bits[1] << 1) | bits[2])


def _compute_mu(x):
    """Global mean of x in f32, matching jnp.mean(x) (XLA CPU) as closely as
    possible.  Integer-exactness is not required; only elements within ~1 ulp
    of mu are at risk, which the caller can check."""
    try:
        import jax
        cpu = jax.devices("cpu")[0]
        import jax.numpy as jnp
        with jax.default_device(cpu):
            mu = np.float32(jnp.mean(jax.device_put(x, cpu)))
    except Exception:
        mu = np.float64(x, copy=False).mean().astype(np.float32)
    return float(mu)


def _host_prep(x, w1, b1, w2, b2, w3, b3):
    import ml_dtypes
    bf16 = ml_dtypes.bfloat16
    x = np.ascontiguousarray(np.asarray(x, dtype=np.float32))
    s1 = _sign1(np.asarray(w1, dtype=np.float32)[:, 0])       # [14, 7, 7]
    rb1 = np.round(np.asarray(b1, dtype=np.float32))          # [14]
    w2b = _sign1(np.asarray(w2, dtype=np.float32)[:, :, 0, 0])  # [128, 14]
    rb2 = np.round(np.asarray(b2, dtype=np.float32))          # [128]
    w3b = _sign1(np.asarray(w3, dtype=np.float32)[:, :, 0, 0])  # [128, 128]
    rb3 = np.round(np.asarray(b3, dtype=np.float32))          # [128]

    mu = _compute_mu(x)

    # border constants
    e0 = _sign1(rb3)                                          # [128]
    c2 = _sign1(rb2)                                          # [128] = y2 at zero-y1
    e1 = _sign1(w3b @ c2 + rb3)                               # [128]

    consts = {
        "combos": np.ascontiguousarray(_build_combos()),
        "w2t": np.ascontiguousarray(w2b.T.astype(bf16)),      # [14, 128]
        "w3t": np.ascontiguousarray(w3b.T.astype(bf16)),      # [128, 128]
        "t2v": np.ascontiguousarray((-rb2).reshape(128, 1).astype(np.float32)),
        "b3v": np.ascontiguousarray((rb3 + 0.25).reshape(128, 1).astype(np.float32)),
        # per-partition-broadcast scalars: col c = round(b1[c]) + 0.25; -mean
        "b1v": np.ascontiguousarray(
            np.tile((rb1 + 0.25).astype(np.float32), (128, 1))),
        "muv": np.full((128, 1), -mu, dtype=np.float32),
        # row 0: all e0.  rows 1/130: [e0, e1 x 130].
        "brow": np.ascontiguousarray(np.repeat(e0.reshape(128, 1), QO, axis=1)),
        "e1row": np.ascontiguousarray(
            np.concatenate([e0.reshape(128, 1),
                            np.repeat(e1.reshape(128, 1), QO - 1, axis=1)], axis=1)),
        # per-4-row border columns for assembled output blocks:
        # cols 0,1 = (e0, e1); col 130 = e1
        "e01q": np.ascontiguousarray(
            np.tile(np.stack([e0, e1], axis=1)[:, None, :], (1, 4, 1))),
        "e130q": np.ascontiguousarray(
            np.repeat(e1.reshape(128, 1), 4, axis=1).reshape(128, 4, 1)),
    }
    return x, s1, rb1, mu, consts


def _build_program(s1, rb1, mu, exact_ties):
    """Build the Bass/Tile program.  s1 = tap signs, rb1 = rounded conv1 bias,
    mu = global mean (baked), exact_ties: use is_ge path for x binarization
    (exact at x == mu) instead of single-op ACT Sign."""
    from contextlib import ExitStack
    import concourse.bacc as bacc
    import concourse.bass as bass
    import concourse.mybir as mybir
    import concourse.tile as tile

    bf16 = mybir.dt.bfloat16
    f32 = mybir.dt.float32
    SIGN = mybir.ActivationFunctionType.Sign
    GE = mybir.AluOpType.is_ge
    MUL = mybir.AluOpType.mult
    ADD = mybir.AluOpType.add

    nc = bacc.Bacc("TRN2", target_bir_lowering=False, debug=False,
                   num_devices=NCORES)

    xk = nc.dram_tensor("x", [C_IN, H, W], f32, kind="ExternalInput")
    combos_d = nc.dram_tensor("combos", [12, NG, NG], bf16, kind="ExternalInput")
    w2t_d = nc.dram_tensor("w2t", [C_IN, 128], bf16, kind="ExternalInput")
    w3t_d = nc.dram_tensor("w3t", [128, 128], bf16, kind="ExternalInput")
    t2v_d = nc.dram_tensor("t2v", [128, 1], f32, kind="ExternalInput")
    b3v_d = nc.dram_tensor("b3v", [128, 1], f32, kind="ExternalInput")
    b1v_d = nc.dram_tensor("b1v", [128, C_IN], f32, kind="ExternalInput")
    muv_d = nc.dram_tensor("muv", [128, 1], f32, kind="ExternalInput")
    brow_d = nc.dram_tensor("brow", [128, QO], f32, kind="ExternalInput")
    e1row_d = nc.dram_tensor("e1row", [128, QO], f32, kind="ExternalInput")
    e01q_d = nc.dram_tensor("e01q", [128, 4, 2], f32, kind="ExternalInput")
    e130q_d = nc.dram_tensor("e130q", [128, 4, 1], f32, kind="ExternalInput")
    out_d = nc.dram_tensor("out", [C_OUT, QO, QO], f32, kind="ExternalOutput")

    with tile.TileContext(nc) as tc, ExitStack() as ctx:
        consts = ctx.enter_context(tc.tile_pool(name="consts", bufs=1))
        stage = ctx.enter_context(tc.tile_pool(name="stage", bufs=4))
        xbp = ctx.enter_context(tc.tile_pool(name="xb", bufs=1))
        y1p = ctx.enter_context(tc.tile_pool(name="y1", bufs=1))
        bigp = ctx.enter_context(tc.tile_pool(name="big", bufs=1))
        y2p = ctx.enter_context(tc.tile_pool(name="y2", bufs=3))
        y3p = ctx.enter_context(tc.tile_pool(name="y3", bufs=3))
        ps1p = ctx.enter_context(tc.tile_pool(name="ps1", bufs=1, space="PSUM"))
        ps2p = ctx.enter_context(tc.tile_pool(name="ps2", bufs=2, space="PSUM"))
        ps3p = ctx.enter_context(tc.tile_pool(name="ps3", bufs=2, space="PSUM"))

        # ---- constant border rows (independent, DRAM->DRAM, contiguous) ----
        nc.gpsimd.dma_start(out_d[:, 0, :], brow_d[:])
        nc.gpsimd.dma_start(out_d[:, 1, :], e1row_d[:])
        nc.gpsimd.dma_start(out_d[:, QO - 1, :], e1row_d[:])

        # ---- load constants into SBUF ----
        combo_t = []
        for i in range(12):
            t = consts.tile([NG, NG], bf16, tag=f"combo{i}")
            nc.sync.dma_start(t[:], combos_d[i])
            combo_t.append(t)
        w2t_t = consts.tile([C_IN, 128], bf16, tag="w2t")
        nc.sync.dma_start(w2t_t[:], w2t_d[:])
        w3t_t = consts.tile([128, 128], bf16, tag="w3t")
        nc.sync.dma_start(w3t_t[:], w3t_d[:])
        t2v_t = consts.tile([128, 1], f32, tag="t2v")
        nc.sync.dma_start(t2v_t[:], t2v_d[:])
        b3v_t = consts.tile([128, 1], f32, tag="b3v")
        nc.sync.dma_start(b3v_t[:], b3v_d[:])
        b1v_t = consts.tile([128, C_IN], f32, tag="b1v")
        nc.sync.dma_start(b1v_t[:], b1v_d[:])
        muv_t = consts.tile([128, 1], f32, tag="muv")
        nc.sync.dma_start(muv_t[:], muv_d[:])
        e01q_t = consts.tile([128, 4, 2], f32, tag="e01q")
        nc.sync.dma_start(e01q_t[:], e01q_d[:])
        e130q_t = consts.tile([128, 4, 1], f32, tag="e130q")
        nc.sync.dma_start(e130q_t[:], e130q_d[:])

        # ---- x load + binarize into phase tiles xb[c][phi] [128, 390] bf16 ----
        # col layout: [3 zero pad | x cols 0..383 | 3 zero pad]
        xb = {}
        for c in range(C_IN):
            for phi in range(3):
                idx = c * 3 + phi
                st = stage.tile([NG, W], f32, tag="xstage")
                nc.sync.dma_start(st[:], xk[c, phi::3, :])
                t = xbp.tile([NG, W + 6], bf16, tag=f"xb{idx}")
                nc.vector.memset(t[:, 0:3], 0.0)
                nc.vector.memset(t[:, W + 3:W + 6], 0.0)
                use_act = (idx % 7) < 4 and not exact_ties
                if use_act:
                    nc.scalar.activation(t[:, 3:W + 3], st[:], SIGN,
                                         bias=muv_t[:])
                else:
                    g = stage.tile([NG, W], bf16, tag="gtmp")
                    nc.vector.tensor_scalar(g[:], st[:], mu, None, GE)
                    nc.vector.tensor_scalar(t[:, 3:W + 3], g[:], 2.0, -1.0, MUL, ADD)
                xb[(c, phi)] = t

        # ---- conv1: depthwise 7x7 at strided sample points via shift-combo
        # matmuls.  psum layout: 4 tiles [128, 512] f32, quarter per channel.
        Y1 = bigp.tile([C_IN, PIX], bf16, tag="Y1")
        psq = [ps1p.tile([NG, 512], f32, tag=f"ps1_{g}", name=f"ps1_{g}")
               for g in range(4)]
        for c in range(C_IN):
            pt = psq[c // 4]
            qoff = (c % 4) * NG
            n_mm = len(_DX_ORDER) * 3
            i_mm = 0
            for dx in _DX_ORDER:
                qj0, qj1 = _DX_QJ[dx]
                for slot in range(3):
                    phi, _ = _SLOTS[slot]
                    ci = _combo_idx(s1, c, slot, dx)
                    rhs = xb[(c, phi)][:, 5 + dx + 3 * qj0: 5 + dx + 3 * qj1: 3]
                    nc.tensor.matmul(
                        pt[:, qoff + qj0: qoff + qj1],
                        combo_t[ci][:],
                        rhs,
                        start=(i_mm == 0),
                        stop=(i_mm == n_mm - 1),
                        skip_group_check=True,
                    )
                    i_mm += 1
            # sign step (+1 iff psum + rb1 >= 0; psum is integer, +0.25 guard)
            y1sb = y1p.tile([NG, NG], bf16, tag=f"y1_{c}")
            nc.scalar.activation(y1sb[:], pt[:, qoff:qoff + NG], SIGN,
                                 bias=b1v_t[:, c:c + 1])
            # flatten [128 qi, 128 qj] -> Y1[c, qi*128+qj]
            nc.sync.dma_start(Y1[c:c + 1, :], y1sb[:])

        # ---- conv2 (1x1, K=14) + conv3 (1x1, K=128) + output, per 512-px block
        for b in range(NBLK):
            sl = slice(b * BLK, (b + 1) * BLK)
            p2 = ps2p.tile([128, BLK], f32)
            nc.tensor.matmul(p2[:], w2t_t[:], Y1[:, sl], start=True, stop=True)
            g2 = y2p.tile([128, BLK], bf16, tag="g2")
            nc.vector.tensor_scalar(g2[:], p2[:], t2v_t[:], None, GE)
            y2sb = y2p.tile([128, BLK], bf16, tag="y2")
            nc.vector.tensor_scalar(y2sb[:], g2[:], 2.0, -1.0, MUL, ADD)
            p3 = ps3p.tile([128, BLK], f32)
            nc.tensor.matmul(p3[:], w3t_t[:], y2sb[:], start=True, stop=True)
            # assemble 4 full 131-wide output rows: border cols + interior
            y3sb = y3p.tile([128, 4, QO], f32, tag="y3")
            nc.vector.tensor_copy(y3sb[:, :, 0:2], e01q_t[:])
            nc.vector.tensor_copy(y3sb[:, :, QO - 1:QO], e130q_t[:])
            nc.scalar.activation(y3sb[:, :, 2:2 + NG], p3[:], SIGN, bias=b3v_t[:])
            r0 = 2 + 4 * b
            nc.gpsimd.dma_start(out_d[:, r0:r0 + 4, :], y3sb[:])

    nc.compile()
    return nc


def _prepare(inputs):
    """Host prep + program build.  Returns (nc, in_maps)."""
    x, s1, rb1, mu, consts = _host_prep(**inputs)
    # is_ge path is exact at x == mu ties; single-op ACT Sign is only safe
    # when no element ties with the mean.
    exact_ties = bool(np.any(x == np.float32(mu)))
    nc = _build_program(s1, rb1, mu, exact_ties)

    in_maps = []
    for k in range(NCORES):
        m = {"x": np.ascontiguousarray(x[k])}
        m.update(consts)
        in_maps.append(m)
    return nc, in_maps


def _run(inputs, trace=False):
    """Build + compile + run on 8 cores.  Returns (out [8,128,131,131], results)."""
    from concourse import bass_utils

    nc, in_maps = _prepare(inputs)

    try:
        res = bass_utils.run_bass_kernel_spmd(
            nc, in_maps, core_ids=list(range(NCORES)), trace=trace)
    except ModuleNotFoundError:
        # axon client without the NTFF profile hook: run without trace
        res = bass_utils.run_bass_kernel_spmd(
            nc, in_maps, core_ids=list(range(NCORES)), trace=False)
    out = np.stack([r["out"] for r in res.results]).astype(np.float32)
    return out, res


def kernel(x, w1, b1, w2, b2, w3, b3):
    out, _ = _run(dict(x=x, w1=w1, b1=b1, w2=w2, b2=b2, w3=w3, b3=b3))
    return out
